# revision 28
# baseline (speedup 1.0000x reference)
"""Conv4d (3,3,3,3) kernel for Trainium2, 8 NeuronCores.

Problem: x (2,24,16,16,48,48) * weight (48,24,3,3,3,3) + bias3d.sum(0)
      -> out (2,48,14,14,46,46), stride 1, no padding.

Strategy
--------
Sharding: 8 cores = (batch 2) x (ol-block 2) x (od-block 2). Each core owns a
7x7 block of (ol, od) output planes (49 tasks).

Per task: implicit GEMM. Contraction rows = (lo, do, ci) = 216 (+1 bias row),
packed on the host into xs[t, 217(pad 256), 48, 48]; row r = (lo*3+do)*24+ci
is the input plane x[b, ci, ol+lo, od+do, :, :], row 216 all-ones. For each
of the 9 (ho, wo) kernel offsets the moving operand is the same SBUF tile
sliced [k, oh0+ho : oh0+ho+rows, wo : wo+46]; offsets accumulate in PSUM.

4-strip col tiling: the 128x128 PE array runs up to FOUR M=32 matmuls
concurrently on disjoint 32-col strips (tile_position (0, 32s)),
microbenchmarked 49.0 ns/MM at N=460 f16 vs 197.5 sequential (4.03x).
Work is cut into M=32 jobs and round-robined over 4 emission lanes, one
per strip:
  - per task pair (tA, tB = consecutive ol): job A32 (co 0:32 of tA,
    18 passes/chunk), job B32 (co 0:32 of tB), and job P (co 32:48 of BOTH
    tasks l-packed into one M=32 = (16 co x 2 planes), 27 passes/chunk
    using only the pair's existing tiles: K-groups k1B, k2B, k1A[0:72]).
  - 63 strip-passes per pair vs 72 for two M=48 streams (-12.5% PE).
Weights for P are block-shifted: the two output planes differ by one in ol,
so plane A's taps against B's rows are W[..., lo+1] (rows shifted by 72).

All k-tile loads are 128-partition DMAs (xs padded to 256 rows): a
128-partition InstDMACopy spreads 8 descriptors to each of 16 SDMA engines,
any other count serializes on one engine (353 vs 175 ns/packet). k1 loads
ride the SP HWDGE queue, k2 loads the Activation queue. Output is staged in
SBUF as bf16 and stored per (job, co-range) via HWDGE.
"""

import os
import sys

if "/opt/trn_rl_repo" not in sys.path:
    sys.path.insert(0, "/opt/trn_rl_repo")

import numpy as np

from concourse import bacc, bass, tile
from concourse.bass_utils import run_bass_kernel_spmd

mybir = bass.mybir

B, CI, CO = 2, 24, 48
L, D, H, W = 16, 16, 48, 48
OL, OD, OH, OW = 14, 14, 46, 46
N_TASKS = 49
KROWS = 217
KSPLIT = 128
K2 = KROWS - KSPLIT  # 89
KG3 = 72  # packed-job group 3: task A's lo=0 rows

CHUNK_ROWS = (10, 10, 10, 10, 6)
CHUNK_OH0 = (0, 10, 20, 30, 40)
NCHUNK = len(CHUNK_ROWS)

DTYPE = mybir.dt.float16
ODTYPE = mybir.dt.bfloat16
X_BUFS = int(os.environ.get("CONV_XBUFS", "6"))
O_BUFS = int(os.environ.get("CONV_OBUFS", "3"))


def build_program(n_tasks: int = N_TASKS, repeat: int = 1):
    from contextlib import nullcontext

    nc = bacc.Bacc()
    f32 = mybir.dt.float32

    xs_d = nc.dram_tensor("xs", [n_tasks, 256, H, W], DTYPE, kind="ExternalInput")
    out_d = nc.dram_tensor("out", [n_tasks, CO, OH, OW], ODTYPE, kind="ExternalOutput")
    w1_d = nc.dram_tensor("w1", [KSPLIT, 9, CO], DTYPE, kind="ExternalInput")
    w2_d = nc.dram_tensor("w2", [K2, 9, CO], DTYPE, kind="ExternalInput")
    wp1_d = nc.dram_tensor("wp1", [KSPLIT, 9, 32], DTYPE, kind="ExternalInput")
    wp2_d = nc.dram_tensor("wp2", [K2, 9, 32], DTYPE, kind="ExternalInput")
    wp3_d = nc.dram_tensor("wp3", [KG3, 9, 32], DTYPE, kind="ExternalInput")

    seq = [0]

    def nm(pfx):
        seq[0] += 1
        return f"{pfx}{seq[0]}"

    with tile.TileContext(nc) as tc:
        with (
            tc.tile_pool(name="wpool", bufs=1) as wpool,
            tc.tile_pool(name="xpool", bufs=X_BUFS) as xpool,
            tc.tile_pool(name="opool", bufs=O_BUFS) as opool,
            tc.tile_pool(name="pspool", bufs=2, space="PSUM") as pspool,
            tc.For_i(0, repeat, 1) if repeat > 1 else nullcontext(),
        ):
            w1s = wpool.tile([KSPLIT, 9, CO], DTYPE)
            w2s = wpool.tile([K2, 9, CO], DTYPE)
            wp1s = wpool.tile([KSPLIT, 9, 32], DTYPE)
            wp2s = wpool.tile([K2, 9, 32], DTYPE)
            wp3s = wpool.tile([KG3, 9, 32], DTYPE)
            nc.sync.dma_start(out=w1s[:], in_=w1_d[:])
            nc.scalar.dma_start(out=w2s[:], in_=w2_d[:])
            nc.sync.dma_start(out=wp1s[:], in_=wp1_d[:])
            nc.scalar.dma_start(out=wp2s[:], in_=wp2_d[:])
            nc.sync.dma_start(out=wp3s[:], in_=wp3_d[:])

            # Per-group (pair or single) resources created lazily at first touch.
            group_res = {}

            def get_group(tA, has_b):
                if tA in group_res:
                    return group_res[tA]
                k1A = xpool.tile([KSPLIT, H, W], DTYPE, name=nm("k1A"), tag="k1A")
                k2A = xpool.tile([KSPLIT, H, W], DTYPE, name=nm("k2A"), tag="k2A")
                nc.sync.dma_start(out=k1A[:], in_=xs_d[tA, 0:KSPLIT])
                nc.scalar.dma_start(out=k2A[:], in_=xs_d[tA, KSPLIT : KSPLIT + 128])
                k1B = k2B = None
                if has_b:
                    k1B = xpool.tile([KSPLIT, H, W], DTYPE, name=nm("k1B"), tag="k1B")
                    k2B = xpool.tile([KSPLIT, H, W], DTYPE, name=nm("k2B"), tag="k2B")
                    nc.sync.dma_start(out=k1B[:], in_=xs_d[tA + 1, 0:KSPLIT])
                    nc.scalar.dma_start(
                        out=k2B[:], in_=xs_d[tA + 1, KSPLIT : KSPLIT + 128]
                    )
                oT = opool.tile([128, OH, OW], ODTYPE, name=nm("oT"), tag="oT")
                group_res[tA] = (k1A, k2A, k1B, k2B, oT)
                return group_res[tA]

            def job_std(s, tA, has_b, task_b, co0, cow):
                """M=cow job at strip s: out[t, co0:co0+cow] via 18 passes/chunk."""
                res = get_group(tA, has_b)
                k1, k2 = (res[2], res[3]) if task_b else (res[0], res[1])
                oT = res[4]
                t = tA + (1 if task_b else 0)
                lo, hi = 32 * s, 32 * s + cow
                for c in range(NCHUNK):
                    rows, oh0 = CHUNK_ROWS[c], CHUNK_OH0[c]
                    n = rows * OW
                    ps = pspool.tile([128, 512], f32, name=nm("ps"), tag=f"ps{s}")
                    for idx in range(9):
                        ho, wo = divmod(idx, 3)
                        for kt, (ks, ws, kr) in enumerate(
                            ((k1, w1s, KSPLIT), (k2, w2s, K2))
                        ):
                            nc.tensor.matmul(
                                ps[lo:hi, 0:n],
                                lhsT=ws[:, idx, co0 : co0 + cow],
                                rhs=ks[0:kr, oh0 + ho : oh0 + ho + rows, wo : wo + OW],
                                start=(idx == 0 and kt == 0),
                                stop=(idx == 8 and kt == 1),
                                tile_position=(0, 32 * s),
                            )
                            yield
                    nc.vector.tensor_copy(
                        out=oT[lo:hi, oh0 : oh0 + rows, :], in_=ps[lo:hi, 0:n]
                    )
                eng = nc.scalar if task_b else nc.sync
                eng.dma_start(out=out_d[t, co0 : co0 + cow], in_=oT[lo:hi])

            def job_packed(s, tA):
                """M=32 job at strip s: co 32:48 of tasks (tA, tA+1) l-packed.

                psum cols [32s : 32s+16] = task A (plane olB-1), [+16 : +32] =
                task B. K-groups per offset: k1B (128 rows), k2B (89, ones row
                feeds bias to both planes), k1A[0:72] (task A's lo=0 rows,
                which are plane olA = olB-1's own first tap).
                """
                k1A, _, k1B, k2B, oT = get_group(tA, True)
                tB = tA + 1
                lo = 32 * s
                for c in range(NCHUNK):
                    rows, oh0 = CHUNK_ROWS[c], CHUNK_OH0[c]
                    n = rows * OW
                    ps = pspool.tile([128, 512], f32, name=nm("ps"), tag=f"ps{s}")
                    for idx in range(9):
                        ho, wo = divmod(idx, 3)
                        for g, (ks, ws, kr) in enumerate(
                            ((k1B, wp1s, KSPLIT), (k2B, wp2s, K2), (k1A, wp3s, KG3))
                        ):
                            nc.tensor.matmul(
                                ps[lo : lo + 32, 0:n],
                                lhsT=ws[:, idx, :],
                                rhs=ks[0:kr, oh0 + ho : oh0 + ho + rows, wo : wo + OW],
                                start=(idx == 0 and g == 0),
                                stop=(idx == 8 and g == 2),
                                tile_position=(0, 32 * s),
                            )
                            yield
                    nc.vector.tensor_copy(
                        out=oT[lo : lo + 32, oh0 : oh0 + rows, :],
                        in_=ps[lo : lo + 32, 0:n],
                    )
                nc.sync.dma_start(out=out_d[tA, 32:CO], in_=oT[lo : lo + 16])
                nc.scalar.dma_start(out=out_d[tB, 32:CO], in_=oT[lo + 16 : lo + 32])

            # Jobs: within each 7-task (d0) column, tasks pair along ol:
            # (0,1),(2,3),(4,5) packed pairs + single task 6 as two M<=32 jobs.
            jobs = []
            assert n_tasks == 49
            for col in range(7):
                for k in range(3):
                    tA = 7 * col + 2 * k
                    jobs.append(lambda s, t=tA: job_std(s, t, True, False, 0, 32))
                    jobs.append(lambda s, t=tA: job_std(s, t, True, True, 0, 32))
                    jobs.append(lambda s, t=tA: job_packed(s, t))
                tS = 7 * col + 6
                jobs.append(lambda s, t=tS: job_std(s, t, False, False, 0, 32))
                jobs.append(lambda s, t=tS: job_std(s, t, False, False, 32, 16))

            lane_q = [[] for _ in range(4)]
            for i, j in enumerate(jobs):
                lane_q[i % 4].append(j)

            def lane_iter(s):
                for jf in lane_q[s]:
                    yield from jf(s)

            iters = [lane_iter(s) for s in range(4)]
            active = [True] * 4
            while any(active):
                for s in range(4):
                    if active[s]:
                        try:
                            next(iters[s])
                        except StopIteration:
                            active[s] = False
    nc.finalize()
    return nc


def make_in_maps(x, weight, bias3d, n_tasks: int = N_TASKS):
    """Host-side shard + repack into the per-task packed-row layout."""
    npdt = mybir.dt.np(DTYPE)
    x = np.asarray(x, np.float32)
    weight = np.asarray(weight, np.float32)
    bias3d = np.asarray(bias3d, np.float32)

    # W[(lo*3+do)*24+ci, ho*3+wo, co] = weight[co, ci, lo, do, ho, wo]
    Wr = np.ascontiguousarray(np.transpose(weight, (2, 3, 1, 4, 5, 0))).reshape(
        216, 9, CO
    )
    Wfull = np.zeros((KROWS, 9, CO), np.float32)
    Wfull[:216] = Wr
    Wfull[216, 0, :] = bias3d.sum(axis=0)
    w1 = np.ascontiguousarray(Wfull[:KSPLIT]).astype(npdt)
    w2 = np.ascontiguousarray(Wfull[KSPLIT:]).astype(npdt)

    # Packed-job weights [K, 9, (p0 co16 | p1 co16)], B-row indexed:
    # p1 (task B, plane olB) uses W rows verbatim; p0 (task A = olB-1) uses
    # taps shifted one l forward: row r -> Wfull[r + 72] (valid lo <= 1).
    # Ones row 216 carries the bias for both planes.
    wp = np.zeros((KROWS, 9, 32), np.float32)
    wp[0:144, :, 0:16] = Wfull[72:216, :, 32:48]
    wp[216, :, 0:16] = Wfull[216, :, 32:48]
    wp[:, :, 16:32] = Wfull[:, :, 32:48]
    wp1 = np.ascontiguousarray(wp[:KSPLIT]).astype(npdt)
    wp2 = np.ascontiguousarray(wp[KSPLIT:]).astype(npdt)
    wp3 = np.zeros((KG3, 9, 32), np.float32)
    wp3[:, :, 0:16] = Wfull[0:KG3, :, 32:48]
    wp3 = wp3.astype(npdt)

    in_maps = []
    for c in range(8):
        b, lb, db = c // 4, (c // 2) % 2, c % 2
        slab = np.ascontiguousarray(
            x[b, :, 7 * lb : 7 * lb + 9, 7 * db : 7 * db + 9]
        )  # (24, 9, 9, 48, 48)
        s_ci, s_l, s_d, s_h, s_w = slab.strides
        # V[l0, d0, lo, do, ci, h, w] = slab[ci, l0+lo, d0+do, h, w]
        V = np.lib.stride_tricks.as_strided(
            slab,
            shape=(7, 7, 3, 3, CI, H, W),
            strides=(s_l, s_d, s_l, s_d, s_ci, s_h, s_w),
        )
        # d0-major task order: task t = d0*7 + l0, so consecutive tasks in a
        # 7-task column are ol-neighbors (what the packed-job weights assume).
        xs = np.zeros((N_TASKS, 256, H, W), np.float32)
        xs[:, :216] = V.transpose(1, 0, 2, 3, 4, 5, 6).reshape(N_TASKS, 216, H, W)
        xs[:, 216] = 1.0
        in_maps.append(
            {
                "xs": xs[:n_tasks].astype(npdt),
                "w1": w1,
                "w2": w2,
                "wp1": wp1,
                "wp2": wp2,
                "wp3": wp3,
            }
        )
    return in_maps


def assemble_output(results):
    out = np.empty((B, CO, OL, OD, OH, OW), np.float32)
    for c in range(8):
        b, lb, db = c // 4, (c // 2) % 2, c % 2
        r = np.asarray(results[c]["out"]).astype(np.float32).reshape(7, 7, CO, OH, OW)
        # r is [d0, l0, ...] (d0-major task order)
        out[b, :, 7 * lb : 7 * lb + 7, 7 * db : 7 * db + 7] = r.transpose(2, 1, 0, 3, 4)
    return out


_NC_CACHE = {}


def _get_program():
    if "nc" not in _NC_CACHE:
        _NC_CACHE["nc"] = build_program()
    return _NC_CACHE["nc"]


def kernel(x, weight, bias3d):
    nc = _get_program()
    in_maps = make_in_maps(x, weight, bias3d)
    res = run_bass_kernel_spmd(nc, in_maps, list(range(8))).results
    return assemble_output(res)
